# revision 26
# baseline (speedup 1.0000x reference)
"""ALSConv GNN layer on 8 Trainium2 NeuronCores.

Distribution: nodes dst-sharded across 8 cores; edge structure known at
compile time, so all irregular indexing becomes static host-built data:
  - each core's dst nodes are greedy-packed into "ptiles" of <=128 nodes so
    every (src-chunk, ptile) edge sub-stream fits TC=256 slots -> identical
    SPMD instruction structure on all cores, per-core content in data.
  - per-edge gathers (k[src], q[dst], z[src] per iteration) use dma_gather
    (SWDGE, int16 indices) from a 4-chunk src table (2*NP<=32767 rows/chunk).
  - segment sums (softmax denom + message aggregation) are TensorE matmuls
    with static one-hot fp8 S matrices accumulated in PSUM.
  - z exchanged per iteration with an AllGather collective.
Softmax skips the segment-max shift (logits are O(few) so exp cannot
overflow in f32); denominator reciprocal is folded into the epilogue.
"""
import math
import numpy as np

import concourse.bass as bass
import concourse.tile as tile
import concourse.mybir as mybir
from concourse import bacc
from concourse.bass_utils import run_bass_kernel_spmd

F32 = mybir.dt.float32
BF16 = mybir.dt.bfloat16
FP8 = mybir.dt.float8e4
I16 = mybir.dt.int16
F16 = mybir.dt.float16

NCORES = 8
DIM = 64
HEADS = 4
HD = 16
ALPHA = 0.1
K_ITERS = 8
TC = 256            # slots per (chunk, ptile) sub-stream
NCHUNK = 4
BT = 16             # ptiles per psum block
S_RES_TILES = 304   # S tiles kept SBUF-resident; rest streamed per iteration
QKV_G = 8           # ptiles per qkv matmul group
STAGES = 3          # debug gating: 1=qkv only, 2=+logits, 3=full


def _pack_core(dst_loc, chunk_of_src, n_loc):
    cnt = np.zeros((n_loc, NCHUNK), dtype=np.int64)
    np.add.at(cnt, (dst_loc, chunk_of_src), 1)
    tot = cnt.sum(1)
    order = np.argsort(-tot, kind="stable")
    bins = []
    for d in order:
        placed = False
        for b in bins:
            if b[0] < 128 and np.all(b[1] + cnt[d] <= TC):
                b[2].append(d)
                b[1] += cnt[d]
                b[0] += 1
                placed = True
                break
        if not placed:
            bins.append([1, cnt[d].copy(), [d]])
    return bins


def host_prep(x, edge_index, edge_attr, Wq, Wk, Wv, We):
    N = x.shape[0]
    shard = N // NCORES
    src_g = np.asarray(edge_index[0], dtype=np.int64)
    dst_g = np.asarray(edge_index[1], dtype=np.int64)
    rank_of = dst_g // shard
    x = np.asarray(x, dtype=np.float32)
    edge_attr = np.asarray(edge_attr, dtype=np.float32)

    packs = []
    for r in range(NCORES):
        m = rank_of == r
        packs.append((m, _pack_core(dst_g[m] - r * shard,
                                    src_g[m] // (2 * shard), shard)))

    NT = max(len(p[1]) for p in packs)
    NP = 128 * NT
    assert 2 * NP <= 32767, f"chunk rows {2*NP} exceed int16 range"
    NB = math.ceil(NT / BT)
    SP = NT * NCHUNK * TC
    T_total = SP // 128
    block_bt = [min(BT, NT - B * BT) for B in range(NB)]

    # slot base for (B, c): layout [B][c][pt_local][TC]
    seg_base = {}
    off = 0
    for B in range(NB):
        for c in range(NCHUNK):
            seg_base[(B, c)] = off
            off += block_bt[B] * TC
    assert off == SP
    # base of ptile pt within its (B,c) segment
    pt_base = np.zeros((NT, NCHUNK), dtype=np.int64)
    for pt in range(NT):
        B, pl = pt // BT, pt % BT
        for c in range(NCHUNK):
            pt_base[pt, c] = seg_base[(B, c)] + pl * TC

    # packed ids
    glob_packed = np.empty(N, dtype=np.int64)
    packed_l = []
    for r in range(NCORES):
        m, bins = packs[r]
        pk = np.full(shard, -1, dtype=np.int64)
        for b_i, b in enumerate(bins):
            for j, d in enumerate(b[2]):
                pk[d] = 128 * b_i + j
        packed_l.append(pk)
        glob_packed[r * shard:(r + 1) * shard] = pk + NP * r

    in_maps = []
    wqkv = np.concatenate([np.asarray(Wq, np.float32), np.asarray(Wk, np.float32),
                           np.asarray(Wv, np.float32)], axis=1)
    web = np.asarray(We, np.float32).astype(np.dtype("bfloat16"))
    for r in range(NCORES):
        m, _ = packs[r]
        pk = packed_l[r]
        src_r = src_g[m]
        dst_r = dst_g[m] - r * shard
        ea_rows = np.nonzero(m)[0]
        chk = src_r // (2 * shard)
        e_pt = pk[dst_r] // 128

        # position within (pt, chunk) group
        key = e_pt * NCHUNK + chk
        order = np.argsort(key, kind="stable")
        ks = key[order]
        grp_start = np.r_[0, np.nonzero(np.diff(ks))[0] + 1]
        starts = np.zeros(len(ks), dtype=np.int64)
        starts[grp_start] = 1
        pos = np.arange(len(ks)) - np.maximum.accumulate(np.where(starts, np.arange(len(ks)), 0))
        slot = pt_base[e_pt[order], chk[order]] + pos

        zidx = np.zeros(SP, dtype=np.int64)   # packed src row in chunk [0,2NP)
        qidx = np.zeros(SP, dtype=np.int64)
        scol = np.full(SP, -1, dtype=np.int64)
        ea_slot = np.full(SP, -1, dtype=np.int64)
        zidx[slot] = glob_packed[src_r[order]] - 2 * NP * chk[order]
        qidx[slot] = pk[dst_r[order]]
        scol[slot] = pk[dst_r[order]] % 128
        ea_slot[slot] = ea_rows[order]
        assert zidx.min() >= 0 and zidx.max() < 2 * NP

        def wrap(flat):
            a = np.zeros((128, SP // 16), dtype=np.int16)
            j = np.arange(SP)
            a[j % 16, j // 16] = flat.astype(np.int16)
            for g in range(1, 8):
                a[g * 16:(g + 1) * 16] = a[:16]
            return a

        v_ = ea_slot >= 0
        eaT = np.zeros((DIM, SP), dtype=np.dtype("bfloat16"))
        eaT[:, v_] = edge_attr[ea_slot[v_]].astype(np.dtype("bfloat16")).T
        jj = np.arange(SP)
        bias = np.zeros((128, T_total), dtype=np.float32)
        # -20 logit shift (= e^-5 on ex) keeps msg=ex*z inside f16 range;
        # softmax is invariant to a shared shift (recip renormalizes).
        bias[jj % 128, jj // 128] = np.where(v_, -20.0, -1e30).astype(np.float32)
        S = np.zeros((128, T_total * 128), dtype=mybir.dt.np(FP8))
        e_ok = np.nonzero(scol >= 0)[0]
        S[e_ok % 128, (e_ok // 128) * 128 + scol[e_ok]] = 1.0

        xT = np.zeros((DIM, NP), dtype=np.float32)
        xloc = np.zeros((NP, DIM), dtype=np.float32)
        xr = x[r * shard:(r + 1) * shard]
        xT[:, pk] = xr.T
        xloc[pk] = xr

        # z table packs two nodes per 256B row; zpair = row, parity masks
        # select the half inside the ex multiply (parity is compile-time).
        par = (zidx % 2).astype(np.float32)
        mA = np.zeros((128, T_total), dtype=np.float32)
        mB = np.zeros((128, T_total), dtype=np.float32)
        mA[jj % 128, jj // 128] = 1.0 - par
        mB[jj % 128, jj // 128] = par
        in_maps.append(dict(xT=xT, xloc=xloc, eaT=eaT, zidx=wrap(zidx),
                            zpair=wrap(zidx // 2), mA=mA, mB=mB,
                            qidx=wrap(qidx), sdat=S, bias=bias,
                            wqkv=wqkv, web=web))
    meta = dict(N=N, shard=shard, NT=NT, NP=NP, NB=NB, SP=SP,
                T_total=T_total, block_bt=block_bt, seg_base=seg_base,
                packed_l=packed_l)
    return in_maps, meta


def build_graph(meta):
    NT, NP, NB, SP, T_total = (meta[k] for k in ("NT", "NP", "NB", "SP", "T_total"))
    block_bt = meta["block_bt"]
    seg_base = meta["seg_base"]
    NTAB = NCORES * NP
    n_res = min(S_RES_TILES, T_total)
    ALL = [list(range(NCORES))]

    nc = bacc.Bacc(None, target_bir_lowering=False)
    xT_d = nc.declare_dram_parameter("xT", [DIM, NP], F32, isOutput=False)
    xloc_d = nc.declare_dram_parameter("xloc", [NP, DIM], F32, isOutput=False)
    eaT_d = nc.declare_dram_parameter("eaT", [DIM, SP], BF16, isOutput=False)
    zidx_d = nc.declare_dram_parameter("zidx", [128, SP // 16], I16, isOutput=False)
    zpair_d = nc.declare_dram_parameter("zpair", [128, SP // 16], I16, isOutput=False)
    mA_d = nc.declare_dram_parameter("mA", [128, T_total], F32, isOutput=False)
    mB_d = nc.declare_dram_parameter("mB", [128, T_total], F32, isOutput=False)
    qidx_d = nc.declare_dram_parameter("qidx", [128, SP // 16], I16, isOutput=False)
    sdat_d = nc.declare_dram_parameter("sdat", [128, T_total * 128], FP8, isOutput=False)
    bias_d = nc.declare_dram_parameter("bias", [128, T_total], F32, isOutput=False)
    wqkv_d = nc.declare_dram_parameter("wqkv", [DIM, 3 * DIM], F32, isOutput=False)
    web_d = nc.declare_dram_parameter("web", [DIM, DIM], BF16, isOutput=False)
    out_d = nc.declare_dram_parameter("out", [NP, DIM], F32, isOutput=True)

    with tile.TileContext(nc) as tc:
        with tc.tile_pool(name="dram", bufs=1, space="DRAM") as dram, \
             tc.tile_pool(name="sb", bufs=1) as sb, \
             tc.tile_pool(name="ps", bufs=1, space="PSUM") as ps:

            # q rows padded to 128 cols so gather elems are 256B (bf16)
            q_dram = dram.tile([NP, 2 * DIM], BF16)
            kv_ag_in = dram.tile([NP, 2 * DIM], BF16)
            kv_tab = dram.tile([NTAB, 2 * DIM], BF16, addr_space="Shared")
            # two nodes per 256B row: [NP/2, 128] per rank, [NTAB/2, 128] table
            z_ag_in = [dram.tile([NP // 2, 2 * DIM], BF16, name=f"zin{i}")
                       for i in range(2)]
            z_tab = [dram.tile([NTAB // 2, 2 * DIM], BF16, addr_space="Shared",
                               name=f"ztab{i}") for i in range(K_ITERS)]

            zidx_sb = sb.tile([128, SP // 16], I16)
            qidx_sb = sb.tile([128, SP // 16], I16)
            bias_sb = sb.tile([128, T_total], F32)
            mA_sb = sb.tile([128, T_total], F32)
            mB_sb = sb.tile([128, T_total], F32)
            ex_sb = sb.tile([128, T_total, 4], F16)
            av_sb = sb.tile([128, NT, DIM], BF16)
            recip_sb = sb.tile([128, NT, 4], F32)
            wqkv_sb = sb.tile([DIM, 3 * DIM], F32)
            web_sb = sb.tile([DIM, DIM], BF16)
            s_res_sb = sb.tile([128, n_res * 128], FP8)

            nc.sync.dma_start(out=zidx_sb[:], in_=zidx_d[:])
            nc.sync.dma_start(out=qidx_sb[:], in_=qidx_d[:])
            nc.sync.dma_start(out=bias_sb[:], in_=bias_d[:])
            nc.sync.dma_start(out=mA_sb[:], in_=mA_d[:])
            nc.sync.dma_start(out=mB_sb[:], in_=mB_d[:])
            nc.sync.dma_start(out=wqkv_sb[:], in_=wqkv_d[:])
            nc.sync.dma_start(out=web_sb[:], in_=web_d[:])
            nc.sync.dma_start(out=s_res_sb[:], in_=sdat_d[:, :n_res * 128])

            MAXT = 2 * BT
            g_bufs = [sb.tile([128, MAXT, 2 * DIM], BF16, name=f"gb{i}")
                      for i in range(3)]
            m_bufs = [sb.tile([128, MAXT, 68], F16, name=f"mb{i}") for i in range(6)]
            ea_buf = sb.tile([DIM, MAXT * 128], BF16)
            lg_bufs = [sb.tile([128, MAXT, 4], F32, name=f"lb{i}") for i in range(2)]
            st_bufs = [sb.tile([128, BT, DIM], F32, name=f"st{i}") for i in range(2)]
            stb_bufs = [sb.tile([128, BT, DIM], BF16, name=f"stb{i}") for i in range(2)]
            xs_bufs = [sb.tile([128, BT, DIM], F32, name=f"xs{i}") for i in range(2)]
            xt_buf = sb.tile([DIM, QKV_G * 128], F32)
            qkv_stg = [sb.tile([128, QKV_G, 256], BF16, name=f"qs{i}")
                       for i in range(2)]
            ss_bufs = [sb.tile([128, MAXT * 128], FP8, name=f"ss{i}") for i in range(4)]
            pp2 = ps.tile([128, 2 * BT, DIM], F32, space="PSUM")
            # 128-col stride: accumulation outputs must not cross PSUM banks
            ps_acc = ps.tile([128, BT, 128], F32, space="PSUM")

            for b in g_bufs:
                nc.vector.memset(b[:], 0.0)
            nc.vector.memset(qkv_stg[0][:], 0.0)
            nc.vector.memset(qkv_stg[1][:], 0.0)

            # ---------------- phase 0a: q/k/v ----------------
            pp_flat = pp2[:].rearrange("p t c -> p (t c)")
            NG = math.ceil(NT / QKV_G)
            for g in range(NG):
                p0 = g * QKV_G
                gn = min(QKV_G, NT - p0)
                nc.sync.dma_start(out=xt_buf[:, :gn * 128],
                                  in_=xT_d[:, p0 * 128:(p0 + gn) * 128])
                stg = qkv_stg[g % 2]
                for pl in range(gn):
                    o = pp_flat[:, pl * 256:pl * 256 + 192]
                    nc.tensor.matmul(out=o, lhsT=xt_buf[:, pl * 128:(pl + 1) * 128],
                                     rhs=wqkv_sb[:], start=True, stop=True)
                    nc.scalar.activation(out=av_sb[:, p0 + pl, :],
                                         in_=pp_flat[:, pl * 256 + 128:pl * 256 + 192],
                                         func=mybir.ActivationFunctionType.Copy,
                                         scale=ALPHA)
                    nc.scalar.copy(out=stg[:, pl, 0:64],
                                   in_=pp_flat[:, pl * 256:pl * 256 + 64])
                    nc.scalar.copy(out=stg[:, pl, 128:256],
                                   in_=pp_flat[:, pl * 256 + 64:pl * 256 + 192])
                nc.sync.dma_start(
                    out=q_dram[p0 * 128:(p0 + gn) * 128, :].rearrange(
                        "(t p) c -> p t c", p=128),
                    in_=stg[:, :gn, 0:128])
                nc.sync.dma_start(
                    out=kv_ag_in[p0 * 128:(p0 + gn) * 128, :].rearrange(
                        "(t p) c -> p t c", p=128),
                    in_=stg[:, :gn, 128:256])
            nc.gpsimd.collective_compute(
                "AllGather", mybir.AluOpType.bypass,
                ins=[kv_ag_in.opt()], outs=[kv_tab.opt()], replica_groups=ALL)

            # ---------------- phase 0b: logits + iteration 1 ----------------
            for B in range(NB):
                bt = block_bt[B]
                nt2 = 2 * bt
                seg_msgs = []
                for c in range(NCHUNK):
                    si = 4 * B + c
                    base = seg_base[(B, c)]
                    nidx = bt * TC
                    t0 = base // 128
                    kvg = g_bufs[si % 3]
                    qg = g_bufs[(si + 1) % 3]
                    nc.gpsimd.dma_gather(
                        out_ap=qg[:, :nt2, :], in_ap=q_dram[:],
                        idxs_ap=qidx_sb[:, base // 16:(base + nidx) // 16],
                        num_idxs=nidx, num_idxs_reg=nidx, elem_size=2 * DIM,
                        single_packet=False)
                    nc.gpsimd.dma_gather(
                        out_ap=kvg[:, :nt2, :],
                        in_ap=kv_tab[2 * NP * c:2 * NP * (c + 1), :],
                        idxs_ap=zidx_sb[:, base // 16:(base + nidx) // 16],
                        num_idxs=nidx, num_idxs_reg=nidx, elem_size=2 * DIM,
                        single_packet=False)
                    nc.sync.dma_start(out=ea_buf[:, :nidx],
                                      in_=eaT_d[:, base:base + nidx])
                    for tt in range(nt2):
                        nc.tensor.matmul(
                            out=pp2[:, tt, :],
                            lhsT=ea_buf[:, tt * 128:(tt + 1) * 128],
                            rhs=web_sb[:], start=True, stop=True)
                    nc.vector.tensor_add(out=kvg[:, :nt2, 0:64],
                                         in0=kvg[:, :nt2, 0:64],
                                         in1=pp2[:, :nt2, :])
                    nc.vector.tensor_mul(out=qg[:, :nt2, 0:64],
                                         in0=qg[:, :nt2, 0:64],
                                         in1=kvg[:, :nt2, 0:64])
                    lg = lg_bufs[si % 2]
                    nc.vector.tensor_reduce(
                        lg[:, :nt2, :],
                        qg[:, :nt2, 0:64].rearrange("p t (h d) -> p t h d", h=4),
                        mybir.AxisListType.X, mybir.AluOpType.add)
                    bb = bias_sb[:, t0:t0 + nt2].rearrange("p (t u) -> p t u", u=1) \
                        .to_broadcast([128, nt2, 4])
                    nc.vector.tensor_add(out=lg[:, :nt2, :], in0=lg[:, :nt2, :],
                                         in1=bb)
                    nc.scalar.activation(out=ex_sb[:, t0:t0 + nt2, :],
                                         in_=lg[:, :nt2, :],
                                         func=mybir.ActivationFunctionType.Exp,
                                         scale=0.25)
                    # iteration-1 messages straight from the gathered v half
                    msg = m_bufs[si % 5]
                    exb = ex_sb[:, t0:t0 + nt2, :] \
                        .rearrange("p t (h u) -> p t h u", u=1) \
                        .to_broadcast([128, nt2, 4, 16])
                    nc.vector.tensor_mul(
                        out=msg[:, :nt2, :64].rearrange("p t (h d) -> p t h d", h=4),
                        in0=kvg[:, :nt2, 64:128].rearrange("p t (h d) -> p t h d", h=4),
                        in1=exb)
                    nc.vector.tensor_copy(out=msg[:, :nt2, 64:68],
                                          in_=ex_sb[:, t0:t0 + nt2, :])
                    str_lo = max(t0, n_res)
                    str_hi = t0 + nt2
                    if str_hi > str_lo:
                        nc.sync.dma_start(
                            out=ss_bufs[c][:, :(str_hi - str_lo) * 128],
                            in_=sdat_d[:, str_lo * 128:str_hi * 128])
                    seg_msgs.append((msg, t0, str_lo))
                _accum_block(nc, ps_acc, seg_msgs, s_res_sb, ss_bufs, bt, n_res, 68)
                b0 = B * BT
                nc.vector.tensor_scalar_add(
                    recip_sb[:, b0:b0 + bt, :], ps_acc[:, :bt, 64:68], 1e-16)
                nc.vector.reciprocal(out=recip_sb[:, b0:b0 + bt, :],
                                     in_=recip_sb[:, b0:b0 + bt, :])
                _epilogue(nc, meta, 1, B, ps_acc, recip_sb, av_sb, st_bufs,
                          stb_bufs, xs_bufs, xloc_d, out_d, z_ag_in)
            # phase-0 indices are no longer needed; reuse zidx_sb for the
            # pair-row indices of the iteration gathers
            nc.sync.dma_start(out=zidx_sb[:], in_=zpair_d[:])
            nc.gpsimd.collective_compute(
                "AllGather", mybir.AluOpType.bypass,
                ins=[z_ag_in[1].opt()], outs=[z_tab[1].opt()],
                replica_groups=ALL)

            # ---------------- iterations 2..K ----------------
            for k in range(2, K_ITERS + 1):
                for B in range(NB):
                    bt = block_bt[B]
                    nt2 = 2 * bt
                    seg_msgs = []
                    for c in range(NCHUNK):
                        si = 4 * B + c
                        base = seg_base[(B, c)]
                        nidx = bt * TC
                        t0 = base // 128
                        g = g_bufs[si % 3]
                        nc.gpsimd.dma_gather(
                            out_ap=g[:, :nt2, :],
                            in_ap=z_tab[k - 1][NP * c:NP * (c + 1), :],
                            idxs_ap=zidx_sb[:, base // 16:(base + nidx) // 16],
                            num_idxs=nidx, num_idxs_reg=nidx, elem_size=2 * DIM,
                            single_packet=False)
                        # parity-select: ex*maskA picks the even half of the
                        # fetched pair, ex*maskB the odd half.
                        lgA = lg_bufs[0]
                        lgB = lg_bufs[1]
                        mAb = mA_sb[:, t0:t0 + nt2] \
                            .rearrange("p (t u) -> p t u", u=1) \
                            .to_broadcast([128, nt2, 4])
                        mBb = mB_sb[:, t0:t0 + nt2] \
                            .rearrange("p (t u) -> p t u", u=1) \
                            .to_broadcast([128, nt2, 4])
                        nc.vector.tensor_mul(out=lgA[:, :nt2, :],
                                             in0=ex_sb[:, t0:t0 + nt2, :], in1=mAb)
                        nc.vector.tensor_mul(out=lgB[:, :nt2, :],
                                             in0=ex_sb[:, t0:t0 + nt2, :], in1=mBb)
                        msg = m_bufs[si % 5]
                        tmp = m_bufs[5]
                        exbA = lgA[:, :nt2, :] \
                            .rearrange("p t (h u) -> p t h u", u=1) \
                            .to_broadcast([128, nt2, 4, 16])
                        exbB = lgB[:, :nt2, :] \
                            .rearrange("p t (h u) -> p t h u", u=1) \
                            .to_broadcast([128, nt2, 4, 16])
                        nc.vector.tensor_mul(
                            out=msg[:, :nt2, :64].rearrange("p t (h d) -> p t h d", h=4),
                            in0=g[:, :nt2, 0:64].rearrange("p t (h d) -> p t h d", h=4),
                            in1=exbA)
                        nc.vector.tensor_mul(
                            out=tmp[:, :nt2, :64].rearrange("p t (h d) -> p t h d", h=4),
                            in0=g[:, :nt2, 64:128].rearrange("p t (h d) -> p t h d", h=4),
                            in1=exbB)
                        nc.vector.tensor_add(out=msg[:, :nt2, :64],
                                             in0=msg[:, :nt2, :64],
                                             in1=tmp[:, :nt2, :64])
                        str_lo = max(t0, n_res)
                        str_hi = t0 + nt2
                        if str_hi > str_lo:
                            nc.sync.dma_start(
                                out=ss_bufs[c][:, :(str_hi - str_lo) * 128],
                                in_=sdat_d[:, str_lo * 128:str_hi * 128])
                        seg_msgs.append((msg, t0, str_lo))
                    _accum_block(nc, ps_acc, seg_msgs, s_res_sb, ss_bufs, bt,
                                 n_res, 64)
                    _epilogue(nc, meta, k, B, ps_acc, recip_sb, av_sb, st_bufs,
                              stb_bufs, xs_bufs, xloc_d, out_d, z_ag_in)
                if k < K_ITERS:
                    nc.gpsimd.collective_compute(
                        "AllGather", mybir.AluOpType.bypass,
                        ins=[z_ag_in[k % 2].opt()],
                        outs=[z_tab[k].opt()], replica_groups=ALL)
    nc.finalize()
    return nc


def _accum_block(nc, ps_acc, seg_msgs, s_res_sb, ss_bufs, bt, n_res, ncols):
    for pl in range(bt):
        for c in range(NCHUNK):
            msg, t0, str_lo = seg_msgs[c]
            for u in range(2):
                tt = 2 * pl + u
                tg = t0 + tt
                if tg < n_res:
                    lhs = s_res_sb[:, tg * 128:(tg + 1) * 128]
                else:
                    lhs = ss_bufs[c][:, (tg - str_lo) * 128:(tg - str_lo + 1) * 128]
                nc.tensor.matmul(
                    out=ps_acc[:, pl, :ncols],
                    lhsT=lhs, rhs=msg[:, tt, :ncols],
                    start=(c == 0 and u == 0),
                    stop=(c == NCHUNK - 1 and u == 1))


def _epilogue(nc, meta, k, B, ps_acc, recip_sb, av_sb, st_bufs, stb_bufs,
              xs_bufs, xloc_d, out_d, z_ag_in):
    bt = meta["block_bt"][B]
    b0 = B * BT
    stg = st_bufs[B % 2]
    rb = recip_sb[:, b0:b0 + bt, :] \
        .rearrange("p t (h u) -> p t h u", u=1) \
        .to_broadcast([128, bt, 4, 16])
    nc.vector.tensor_mul(
        out=stg[:, :bt, :].rearrange("p t (h d) -> p t h d", h=4),
        in0=ps_acc[:, :bt, :64].rearrange("p t (h d) -> p t h d", h=4),
        in1=rb)
    zo = stb_bufs[B % 2] if k < K_ITERS else stg
    nc.vector.scalar_tensor_tensor(
        out=zo[:, :bt, :], in0=stg[:, :bt, :], scalar=1.0 - ALPHA,
        in1=av_sb[:, b0:b0 + bt, :],
        op0=mybir.AluOpType.mult, op1=mybir.AluOpType.add)
    if k < K_ITERS:
        zv = z_ag_in[k % 2][:].rearrange("a (b c) -> (a b) c", b=2)
        nc.scalar.dma_start(
            out=zv[b0 * 128:(b0 + bt) * 128, :]
            .rearrange("(t p) c -> p t c", p=128),
            in_=zo[:, :bt, :])
    else:
        xs = xs_bufs[B % 2]
        nc.sync.dma_start(
            in_=xloc_d[b0 * 128:(b0 + bt) * 128, :]
            .rearrange("(t p) c -> p t c", p=128),
            out=xs[:, :bt, :])
        nc.scalar.activation(out=stg[:, :bt, :], in_=stg[:, :bt, :],
                             func=mybir.ActivationFunctionType.Relu)
        nc.vector.tensor_add(out=stg[:, :bt, :], in0=stg[:, :bt, :],
                             in1=xs[:, :bt, :])
        nc.sync.dma_start(
            out=out_d[b0 * 128:(b0 + bt) * 128, :]
            .rearrange("(t p) c -> p t c", p=128),
            in_=stg[:, :bt, :])


LAST = {}


def _timed_run(nc, in_maps, n_iter=24):
    """Steady-state per-execution wall time of the NEFF: pre-stage inputs on
    the 8 devices, then dispatch n_iter back-to-back executions."""
    import time
    import jax
    from jax.sharding import Mesh, PartitionSpec, NamedSharding
    from jax.experimental.shard_map import shard_map
    import concourse.mybir as mybir_
    from concourse import bass2jax
    from concourse.bass2jax import _bass_exec_p, partition_id_tensor

    bass2jax.install_neuronx_cc_hook()
    partition_name = nc.partition_id_tensor.name if nc.partition_id_tensor else None
    in_names, out_names, out_avals, zero_outs = [], [], [], []
    for alloc in nc.m.functions[0].allocations:
        if not isinstance(alloc, mybir_.MemoryLocationSet):
            continue
        name = alloc.memorylocations[0].name
        if alloc.kind == "ExternalInput":
            if name != partition_name:
                in_names.append(name)
        elif alloc.kind == "ExternalOutput":
            out_names.append(name)
            shape = tuple(alloc.tensor_shape)
            dtype = mybir_.dt.np(alloc.dtype)
            out_avals.append(jax.core.ShapedArray(shape, dtype))
            zero_outs.append(np.zeros(shape, dtype))
    n_params = len(in_names)
    all_in = in_names + out_names + ([partition_name] if partition_name else [])

    def _body(*args):
        operands = list(args)
        if partition_name is not None:
            operands.append(partition_id_tensor())
        outs = _bass_exec_p.bind(
            *operands, out_avals=tuple(out_avals), in_names=tuple(all_in),
            out_names=tuple(out_names), lowering_input_output_aliases=(),
            sim_require_finite=True, sim_require_nnan=True, nc=nc)
        return tuple(outs)

    devices = jax.devices()[:NCORES]
    mesh = Mesh(np.asarray(devices), ("core",))
    spec = NamedSharding(mesh, PartitionSpec("core"))
    n_outs = len(out_names)
    sharded = jax.jit(
        shard_map(_body, mesh=mesh,
                  in_specs=(PartitionSpec("core"),) * (n_params + n_outs),
                  out_specs=(PartitionSpec("core"),) * n_outs,
                  check_rep=False),
        keep_unused=True)
    per_core = [[np.asarray(m[n]) for n in in_names] for m in in_maps]
    xin = [jax.device_put(
        np.concatenate([per_core[c][i] for c in range(NCORES)], 0), spec)
        for i in range(n_params)]
    zin = [jax.device_put(
        np.zeros((NCORES * z.shape[0], *z.shape[1:]), z.dtype), spec)
        for z in zero_outs]
    r = sharded(*xin, *zin)
    jax.block_until_ready(r)
    t0 = time.time()
    for _ in range(n_iter):
        r = sharded(*xin, *zin)
    jax.block_until_ready(r)
    t1 = time.time()
    return (t1 - t0) / n_iter * 1e9


def kernel(x, edge_index, edge_attr, Wq, Wk, Wv, We, _time=False):
    import time
    t0 = time.time()
    in_maps, meta = host_prep(x, edge_index, edge_attr, Wq, Wk, Wv, We)
    t1 = time.time()
    nc = build_graph(meta)
    t2 = time.time()
    res = run_bass_kernel_spmd(nc, in_maps, core_ids=list(range(NCORES)))
    t3 = time.time()
    exec_ns = None
    if _time:
        exec_ns = _timed_run(nc, in_maps)
    LAST.update(meta=meta, exec_time_ns=exec_ns, res=res,
                t_prep=t1 - t0, t_build=t2 - t1, t_run=t3 - t2)
    N, shard = meta["N"], meta["shard"]
    out = np.empty((N, DIM), dtype=np.float32)
    for r in range(NCORES):
        pk = meta["packed_l"][r]
        out[r * shard:(r + 1) * shard] = res.results[r]["out"][pk]
    return out

